# revision 12
# baseline (speedup 1.0000x reference)
"""GATv2 2-layer GNN + global mean pool on 8 TRN2 NeuronCores (Bass/Tile).

Host: graph partitioning + metadata in numpy. Device: SPMD kernel on cores
0-7 via run_bass_kernel_spmd. See transcript design notes.

Sharding: core c owns nodes [c*6250, (c+1)*6250) and all edges whose dst is
in that range (self-loops included). Per layer, each core computes its
xl = x@Wl shard (bf16 rows padded to 256 cols), AllGathers the full table,
keeps xr = x@Wr local (row col 192 = 1.0 for the softmax denominator).
Per-edge s = xl[src]+xr[dst] via two bulk dma_gathers + DVE add; leaky_relu
via fused scalar_tensor_tensor; per-head logits via att-broadcast multiply +
strided tree reduce; exp on ScalarE; segment softmax + aggregation fused into
per-chunk TensorE matmuls (lhsT = Sel01*exp) accumulating into a PSUM node
grid (40 nodes x 3 heads = 120 rows per bank, 4 banks = 160 nodes per set).
Normalize by 1/den, write slot-major, reshape to node-major via affine DMAs,
apply -xr + b, relu. Pool via one-hot matmuls + AllReduce; final linear +
softmax replicated on every core.
"""
import sys

sys.path.insert(0, "/opt/trn_rl_repo")

import numpy as np
import ml_dtypes

import concourse.bass as bass
import concourse.mybir as mybir
import concourse.tile as tile
import concourse.bacc as bacc
from concourse import bass_utils
from concourse.masks import make_identity

BF16 = mybir.dt.bfloat16
F32 = mybir.dt.float32
I16 = mybir.dt.int16

N, E, F, H, C, G, NCLS = 50000, 800000, 128, 3, 64, 16, 10
NCORES = 8
NLOC = N // NCORES            # 6250
HALF = N // 2                 # 25000
BANK_NODES = 40               # nodes per PSUM bank -> m = 120
NBANKS = 4                    # PSUM banks used by aggregation per set
SET_NODES = BANK_NODES * NBANKS   # 160
NSETS = -(-NLOC // SET_NODES)     # 40
DPAD = 256                    # padded table row (bf16) -> 512B
DW = 193                      # 192 feats + ones col
NPAD = NSETS * SET_NODES      # 6400
MTILES = -(-NPAD // 128)      # 50


# ------------------------------------------------------------------
# host preprocessing
# ------------------------------------------------------------------

def _wrap16(seq):
    n = seq.size
    w = np.asarray(seq, np.int16).reshape(n // 16, 16).T
    return np.ascontiguousarray(np.tile(w, (8, 1)))


def _preprocess(edge_index, batch):
    src_g = np.concatenate([np.asarray(edge_index[0]), np.arange(N, dtype=np.int64)])
    dst_g = np.concatenate([np.asarray(edge_index[1]), np.arange(N, dtype=np.int64)])

    per_core = []
    counts = np.zeros((NCORES, NSETS, NBANKS, 2), np.int64)
    for c in range(NCORES):
        m = (dst_g // NLOC) == c
        src = src_g[m].astype(np.int64)
        dst = (dst_g[m] - c * NLOC).astype(np.int64)
        half = (src >= HALF).astype(np.int64)
        order = np.argsort(dst * 2 + half, kind="stable")
        src, dst, half = src[order], dst[order], half[order]
        bank_id = dst // BANK_NODES
        set_id = bank_id // NBANKS
        bank = bank_id % NBANKS
        np.add.at(counts[c], (set_id, bank, half), 1)
        per_core.append((src, dst, set_id, bank, half))

    kch = np.maximum(1, -(-counts.max(axis=0) // 128))   # [NSETS, NBANKS, 2]
    reg_ch = kch.sum(axis=1)                             # chunks per (set, half)
    reg_ch += reg_ch % 2                                 # 256-slot alignment
    tot_ch = int(reg_ch.sum())
    tot_slots = tot_ch * 128

    jobs = [[] for _ in range(NSETS)]
    chunk_meta = []          # global chunk -> (set, half, bank or -1)
    set_nch = []
    reg_off = []
    pos_slots = 0
    for k in range(NSETS):
        col = 0
        first = [True] * NBANKS
        offs = []
        for hf in range(2):
            a = pos_slots
            used = 0
            for b in range(NBANKS):
                for _ in range(int(kch[k, b, hf])):
                    jobs[k].append([col, b, first[b], False])
                    first[b] = False
                    chunk_meta.append((k, hf, b))
                    col += 1
                    used += 1
            while used < int(reg_ch[k, hf]):
                chunk_meta.append((k, hf, -1))
                col += 1
                used += 1
            pos_slots += int(reg_ch[k, hf]) * 128
            offs.append((a, pos_slots))
        lastj = {}
        for j, jb in enumerate(jobs[k]):
            lastj[jb[1]] = j
        for b, j in lastj.items():
            jobs[k][j][3] = True
        set_nch.append(col)
        reg_off.append(offs)
    assert sum(set_nch) == tot_ch
    assert pos_slots == tot_slots

    cores = []
    for c in range(NCORES):
        src, dst, set_id, bank, half = per_core[c]
        xl_idx = np.zeros(tot_slots, np.int64)
        xr_idx = np.zeros(tot_slots, np.int64)
        selpat = np.full(tot_slots, -1, np.int64)
        cursor = {}
        pos = 0
        for (k, hf, b) in chunk_meta:
            if b >= 0:
                key = (k, b, hf)
                if key not in cursor:
                    selm = (set_id == k) & (bank == b) & (half == hf)
                    cursor[key] = [src[selm], dst[selm], 0]
                es_all, ed_all, cpos = cursor[key]
                n = min(128, es_all.size - cpos)
                es = es_all[cpos:cpos + n]
                ed = ed_all[cpos:cpos + n]
                cursor[key][2] = cpos + n
                sl = slice(pos, pos + n)
                xl_idx[sl] = es - HALF * hf
                xr_idx[sl] = ed
                selpat[sl] = ed % BANK_NODES
            pos += 128
        for key, (es_all, ed_all, cpos) in cursor.items():
            assert cpos == es_all.size, (c, key, cpos, es_all.size)

        sel = np.zeros((128, tot_ch, 120), ml_dtypes.bfloat16)
        sp = selpat.reshape(tot_ch, 128)
        ii, jj = (sp >= 0).nonzero()     # ii = chunk, jj = row
        for hh in range(H):
            sel[jj, ii, BANK_NODES * hh + sp[ii, jj]] = 1.0
        cores.append(dict(
            xl_idx16=_wrap16(xl_idx), xr_idx16=_wrap16(xr_idx),
            sel01=np.ascontiguousarray(sel.reshape(128, tot_ch * 120))))

    meta = dict(jobs=jobs, set_nch=set_nch, reg_off=reg_off,
                tot_ch=tot_ch, tot_slots=tot_slots)
    return cores, meta


def _onehots(batch, core):
    oh = np.zeros((128, MTILES, G), ml_dtypes.bfloat16)
    base = core * NLOC
    bat = np.asarray(batch, np.int64)
    for t in range(MTILES):
        n0 = t * 128
        n1 = min(n0 + 128, NLOC)
        if n1 > n0:
            rows = np.arange(n0, n1)
            oh[rows - n0, t, bat[base + rows]] = 1.0
    return np.ascontiguousarray(oh.reshape(128, MTILES * G))


# ------------------------------------------------------------------
# device builder
# ------------------------------------------------------------------

def _build(meta):
    nc = bacc.Bacc()
    jobs, set_nch, reg_off = meta["jobs"], meta["set_nch"], meta["reg_off"]
    tot_ch, tot_slots = meta["tot_ch"], meta["tot_slots"]

    xT = nc.declare_dram_parameter("xT", [F, NLOC], BF16, isOutput=False)
    wl1 = nc.declare_dram_parameter("wl1", [F, DPAD], BF16, isOutput=False)
    wr1 = nc.declare_dram_parameter("wr1", [F, DPAD], BF16, isOutput=False)
    wl2 = nc.declare_dram_parameter("wl2", [192, DPAD], BF16, isOutput=False)
    wr2 = nc.declare_dram_parameter("wr2", [192, DPAD], BF16, isOutput=False)
    att1_rep = nc.declare_dram_parameter("att1_rep", [128, 192], BF16, isOutput=False)
    att2_rep = nc.declare_dram_parameter("att2_rep", [128, 192], BF16, isOutput=False)
    b1_rep = nc.declare_dram_parameter("b1_rep", [128, 192], BF16, isOutput=False)
    b2_rep = nc.declare_dram_parameter("b2_rep", [128, 64], F32, isOutput=False)
    wc_in = nc.declare_dram_parameter("wc", [C, NCLS], F32, isOutput=False)
    bc_rep = nc.declare_dram_parameter("bc_rep", [G, NCLS], F32, isOutput=False)
    cntr = nc.declare_dram_parameter("cnt_recip", [G, 1], F32, isOutput=False)
    oh_in = nc.declare_dram_parameter("oh", [128, MTILES * G], BF16, isOutput=False)
    xl_idx = nc.declare_dram_parameter("xl_idx16", [128, tot_slots // 16], I16, isOutput=False)
    xr_idx = nc.declare_dram_parameter("xr_idx16", [128, tot_slots // 16], I16, isOutput=False)
    sel_in = nc.declare_dram_parameter("sel01", [128, tot_ch * 120], BF16, isOutput=False)
    out_ext = nc.declare_dram_parameter("out", [G, NCLS], F32, isOutput=True)

    shard_tab = nc.dram_tensor("shard_tab", [NLOC, DPAD], BF16)
    glob_tab = nc.dram_tensor("glob_tab", [N, DPAD], BF16)
    glob_hi = nc.dram_tensor("glob_hi", [HALF, DPAD], BF16)
    xr_tab = nc.dram_tensor("xr_tab", [NLOC, DPAD], BF16)
    h_slots = nc.dram_tensor("h_slots", [NSETS, 128, NBANKS * DW], BF16)
    h1_node = nc.dram_tensor("h1_node", [NPAD, 192], BF16)
    o2_node = nc.dram_tensor("o2_node", [NPAD, H * 64], BF16)
    pool_in = nc.dram_tensor("pool_in", [G, C], F32)
    pool_out = nc.dram_tensor("pool_out", [G, C], F32)

    with nc.allow_low_precision(reason="bf16 tree-reduce + staging validated within 2e-2 tolerance"), tile.TileContext(nc) as tc:
        with (
            tc.tile_pool(name="const", bufs=1) as cpool,
            tc.tile_pool(name="sbuf", bufs=2) as sb,
            tc.tile_pool(name="agg", bufs=1, space="PSUM") as ps_agg,
            tc.tile_pool(name="ptf", bufs=1, space="PSUM") as ps_tf,
            tc.tile_pool(name="pmisc", bufs=1, space="PSUM") as ps_misc,
            tc.tile_pool(name="big", bufs=1) as mp,
        ):
            t_att1 = cpool.tile([128, 192], BF16, name="t_att1")
            t_att2 = cpool.tile([128, 192], BF16, name="t_att2")
            t_b1 = cpool.tile([128, 192], BF16, name="t_b1")
            t_b2 = cpool.tile([128, 64], F32, name="t_b2")
            t_oh = cpool.tile([128, MTILES * G], BF16, name="t_oh")
            ident = cpool.tile([128, 128], BF16, name="ident")
            nc.sync.dma_start(out=t_att1[:], in_=att1_rep[:])
            nc.sync.dma_start(out=t_att2[:], in_=att2_rep[:])
            nc.sync.dma_start(out=t_b1[:], in_=b1_rep[:])
            nc.sync.dma_start(out=t_b2[:], in_=b2_rep[:])
            nc.sync.dma_start(out=t_oh[:], in_=oh_in[:])
            make_identity(nc, ident[:])

            # ------- layer-1 transforms -------
            t_xT = mp.tile([128, NLOC], BF16, name="t_xT")
            nc.sync.dma_start(out=t_xT[:], in_=xT[:])
            t_wl = cpool.tile([128, DPAD], BF16, name="t_wl")
            t_wr = cpool.tile([128, DPAD], BF16, name="t_wr")
            nc.sync.dma_start(out=t_wl[:], in_=wl1[:])
            nc.sync.dma_start(out=t_wr[:], in_=wr1[:])

            ntile = -(-NLOC // 128)
            for t in range(ntile):
                mr = min(128, NLOC - t * 128)
                for which, (wt, dtab) in enumerate(((t_wl, shard_tab), (t_wr, xr_tab))):
                    pst = ps_tf.tile([128, DPAD], F32, tag="tf", name=f"p1_{t}_{which}")
                    nc.tensor.matmul(pst[0:mr, :],
                                     lhsT=t_xT[:, t * 128:t * 128 + mr],
                                     rhs=wt[:], start=True, stop=True)
                    stg = sb.tile([128, DPAD], BF16, tag="tfs", name=f"s1_{t}_{which}")
                    nc.scalar.copy(out=stg[0:mr, :], in_=pst[0:mr, :])
                    if which == 1:
                        nc.vector.memset(stg[0:mr, 192:193], 1.0)
                    nc.sync.dma_start(out=dtab[t * 128:t * 128 + mr, :],
                                      in_=stg[0:mr, :])

            nc.gpsimd.collective_compute(
                "AllGather", mybir.AluOpType.bypass,
                replica_groups=[list(range(NCORES))],
                ins=[shard_tab[:].opt()], outs=[glob_tab[:].opt()])
            nc.sync.dma_start(out=glob_hi[:], in_=glob_tab[HALF:N, :])

            # ------- edge layer -------
            def edge_layer(layer, t_att):
                for k in range(NSETS):
                    nch = set_nch[k]
                    (lo_a, lo_b), (hi_a, hi_b) = reg_off[k]
                    nsl = nch * 128
                    nlo = lo_b - lo_a
                    nhi = hi_b - hi_a
                    ti_xl = sb.tile([128, nsl // 16], I16, tag="ixl", name=f"ixl{layer}_{k}")
                    ti_xr = sb.tile([128, nsl // 16], I16, tag="ixr", name=f"ixr{layer}_{k}")
                    nc.sync.dma_start(out=ti_xl[:],
                                      in_=xl_idx[:, lo_a // 16:lo_a // 16 + nsl // 16])
                    nc.sync.dma_start(out=ti_xr[:],
                                      in_=xr_idx[:, lo_a // 16:lo_a // 16 + nsl // 16])
                    g_xl = sb.tile([128, nch, DPAD], BF16, tag="gxl", name=f"gxl{layer}_{k}")
                    g_xr = sb.tile([128, nch, DPAD], BF16, tag="gxr", name=f"gxr{layer}_{k}")
                    if nlo > 0:
                        nc.gpsimd.dma_gather(
                            out_ap=g_xl[:, 0:nlo // 128, :],
                            in_ap=glob_tab[0:HALF, :],
                            idxs_ap=ti_xl[:, 0:nlo // 16],
                            num_idxs=nlo, num_idxs_reg=nlo, elem_size=DPAD, single_packet=False)
                    if nhi > 0:
                        nc.gpsimd.dma_gather(
                            out_ap=g_xl[:, nlo // 128:nch, :],
                            in_ap=glob_hi[:],
                            idxs_ap=ti_xl[:, nlo // 16:nsl // 16],
                            num_idxs=nhi, num_idxs_reg=nhi, elem_size=DPAD, single_packet=False)
                    nc.gpsimd.dma_gather(
                        out_ap=g_xr[:], in_ap=xr_tab[:], idxs_ap=ti_xr[:],
                        num_idxs=nsl, num_idxs_reg=nsl, elem_size=DPAD, single_packet=False)
                    t_s = sb.tile([128, nch, DW], BF16, tag="ts", name=f"ts{layer}_{k}")
                    nc.vector.tensor_tensor(out=t_s[:], in0=g_xl[:, :, 0:DW],
                                            in1=g_xr[:, :, 0:DW],
                                            op=mybir.AluOpType.add)
                    t_lk = sb.tile([128, nch, 192], BF16, tag="tlk", name=f"tlk{layer}_{k}")
                    nc.vector.scalar_tensor_tensor(
                        out=t_lk[:], in0=t_s[:, :, 0:192], scalar=0.2,
                        in1=t_s[:, :, 0:192],
                        op0=mybir.AluOpType.mult, op1=mybir.AluOpType.max)
                    att_b = bass.AP(t_att[:].tensor, t_att[:].offset,
                                    [list(t_att[:].ap[0]), [0, nch], [1, 192]])
                    nc.vector.tensor_tensor(out=t_lk[:], in0=t_lk[:], in1=att_b,
                                            op=mybir.AluOpType.mult)
                    v = t_lk[:].rearrange("p c (h w) -> p c h w", h=H)
                    w = 32
                    while w > 1:
                        nc.vector.tensor_tensor(out=v[:, :, :, 0:w],
                                                in0=v[:, :, :, 0:w],
                                                in1=v[:, :, :, w:2 * w],
                                                op=mybir.AluOpType.add)
                        w //= 2
                    t_lg = sb.tile([128, nch, H], F32, tag="tlg", name=f"tlg{layer}_{k}")
                    nc.vector.tensor_tensor(out=t_lg[:],
                                            in0=v[:, :, :, 0:1].squeeze(3),
                                            in1=v[:, :, :, 1:2].squeeze(3),
                                            op=mybir.AluOpType.add)
                    t_e = sb.tile([128, nch, H], BF16, tag="te", name=f"te{layer}_{k}")
                    nc.scalar.activation(out=t_e[:], in_=t_lg[:],
                                         func=mybir.ActivationFunctionType.Exp)
                    t_sel = sb.tile([128, nch, 120], BF16, tag="tsel", name=f"tsel{layer}_{k}")
                    ch0 = sum(set_nch[:k])
                    nc.sync.dma_start(out=t_sel[:],
                                      in_=sel_in[:, ch0 * 120:(ch0 + nch) * 120])
                    eb = bass.AP(t_e[:].tensor, t_e[:].offset,
                                 [list(t_e[:].ap[0]), [H, nch], [1, H], [0, BANK_NODES]])
                    nc.vector.tensor_tensor(
                        out=t_sel[:].rearrange("p c (h s) -> p c h s", h=H),
                        in0=t_sel[:].rearrange("p c (h s) -> p c h s", h=H),
                        in1=eb, op=mybir.AluOpType.mult)
                    pagg = ps_agg.tile([128, NBANKS * 512], F32, tag="pagg",
                                       name=f"pagg{layer}_{k}")
                    for (col, b, st, sp_) in jobs[k]:
                        nc.tensor.matmul(
                            pagg[0:120, b * 512:b * 512 + DW],
                            lhsT=t_sel[:, col, :],
                            rhs=t_s[:, col, :],
                            start=st, stop=sp_)
                    t_ev = sb.tile([128, NBANKS, DW], BF16, tag="tev", name=f"tev{layer}_{k}")
                    pagg_v = bass.AP(pagg[:].tensor, pagg[:].offset,
                                     [list(pagg[:].ap[0]), [512, NBANKS], [1, DW]])
                    nc.scalar.copy(out=t_ev[:], in_=pagg_v)
                    t_d = sb.tile([128, NBANKS], BF16, tag="td", name=f"td{layer}_{k}")
                    nc.vector.reciprocal(out=t_d[:], in_=t_ev[:, :, 192:193].squeeze(2))
                    db = bass.AP(t_d[:].tensor, t_d[:].offset,
                                 [list(t_d[:].ap[0]), [1, NBANKS], [0, DW]])
                    nc.vector.tensor_tensor(out=t_ev[:], in0=t_ev[:], in1=db,
                                            op=mybir.AluOpType.mult)
                    nc.sync.dma_start(out=h_slots[k, :, :],
                                      in_=t_ev[:].rearrange("p b d -> p (b d)"))

            edge_layer(1, t_att1)

            # ------- reshape slots -> node-major (layer 1) -------
            # slot row 40h+s of (set k, bank b) -> node k*160 + b*40 + s,
            # cols [64h, 64h+64)
            for hh in range(H):
                for b in range(NBANKS):
                    srcv = h_slots[:, 40 * hh:40 * hh + 40,
                                   b * DW + 64 * hh:b * DW + 64 * hh + 64]
                    dstv = h1_node[:].rearrange(
                        "(k b s) d -> k b s d", k=NSETS, b=NBANKS)[
                        :, b, :, 64 * hh:64 * hh + 64]
                    nc.sync.dma_start(out=dstv, in_=srcv)

            # ------- h1 = relu(slots/den - xr1 + b1); build h1T planes -------
            t_wl2a = cpool.tile([128, DPAD], BF16, name="t_wl2a")
            t_wl2b = cpool.tile([64, DPAD], BF16, name="t_wl2b")
            t_wr2a = cpool.tile([128, DPAD], BF16, name="t_wr2a")
            t_wr2b = cpool.tile([64, DPAD], BF16, name="t_wr2b")
            nc.sync.dma_start(out=t_wl2a[:], in_=wl2[0:128, :])
            nc.sync.dma_start(out=t_wl2b[:], in_=wl2[128:192, :])
            nc.sync.dma_start(out=t_wr2a[:], in_=wr2[0:128, :])
            nc.sync.dma_start(out=t_wr2b[:], in_=wr2[128:192, :])
            h1T_a = mp.tile([128, NPAD], BF16, name="h1T_a")
            h1T_b = mp.tile([64, NPAD], BF16, name="h1T_b")

            for t in range(MTILES):
                mr = max(0, min(128, NLOC - t * 128))
                t_h = sb.tile([128, 192], BF16, tag="th", name=f"th{t}")
                nc.sync.dma_start(out=t_h[:], in_=h1_node[t * 128:(t + 1) * 128, :])
                th2 = sb.tile([128, 192], BF16, tag="th2", name=f"th2{t}")
                if mr < 128:
                    nc.vector.memset(th2[:], 0.0)
                if mr > 0:
                    t_xr1 = sb.tile([128, DPAD], BF16, tag="txr1", name=f"txr1{t}")
                    nc.sync.dma_start(out=t_xr1[0:mr, :],
                                      in_=xr_tab[t * 128:t * 128 + mr, :])
                    nc.vector.tensor_tensor(out=th2[0:mr, :], in0=t_h[0:mr, :],
                                            in1=t_xr1[0:mr, 0:192],
                                            op=mybir.AluOpType.subtract)
                    nc.vector.tensor_tensor(out=th2[0:mr, :], in0=th2[0:mr, :],
                                            in1=t_b1[0:mr, :],
                                            op=mybir.AluOpType.add)
                    nc.vector.tensor_scalar_max(th2[0:mr, :], th2[0:mr, :], 0.0)
                for h2 in range(2):
                    wdt = 128 if h2 == 0 else 64
                    ptr = ps_misc.tile([128, 512], BF16, tag="pm", name=f"ptr{t}_{h2}")
                    nc.tensor.transpose(out=ptr[0:wdt, 0:128],
                                        in_=th2[:, h2 * 128:h2 * 128 + wdt],
                                        identity=ident[:])
                    dst = h1T_a if h2 == 0 else h1T_b
                    nc.vector.tensor_copy(out=dst[0:wdt, t * 128:(t + 1) * 128],
                                          in_=ptr[0:wdt, 0:128])

            # ------- layer-2 transforms -------
            for t in range(MTILES):
                mr = max(0, min(128, NLOC - t * 128))
                if mr == 0:
                    continue
                for which, (wta, wtb, dtab) in enumerate((
                        (t_wl2a, t_wl2b, shard_tab), (t_wr2a, t_wr2b, xr_tab))):
                    pst = ps_tf.tile([128, DPAD], F32, tag="tf", name=f"p2_{t}_{which}")
                    nc.tensor.matmul(pst[0:mr, :],
                                     lhsT=h1T_a[:, t * 128:t * 128 + mr],
                                     rhs=wta[:], start=True, stop=False)
                    nc.tensor.matmul(pst[0:mr, :],
                                     lhsT=h1T_b[:, t * 128:t * 128 + mr],
                                     rhs=wtb[:], start=False, stop=True)
                    stg = sb.tile([128, DPAD], BF16, tag="tfs", name=f"s2_{t}_{which}")
                    nc.scalar.copy(out=stg[0:mr, :], in_=pst[0:mr, :])
                    if which == 1:
                        nc.vector.memset(stg[0:mr, 192:193], 1.0)
                    nc.sync.dma_start(out=dtab[t * 128:t * 128 + mr, :],
                                      in_=stg[0:mr, :])

            nc.gpsimd.collective_compute(
                "AllGather", mybir.AluOpType.bypass,
                replica_groups=[list(range(NCORES))],
                ins=[shard_tab[:].opt()], outs=[glob_tab[:].opt()])
            nc.sync.dma_start(out=glob_hi[:], in_=glob_tab[HALF:N, :])

            edge_layer(2, t_att2)

            for hh in range(H):
                for b in range(NBANKS):
                    srcv = h_slots[:, 40 * hh:40 * hh + 40,
                                   b * DW + 64 * hh:b * DW + 64 * hh + 64]
                    dstv = o2_node[:].rearrange(
                        "(k b s) (h d) -> k b s h d", k=NSETS, b=NBANKS, h=H)[
                        :, b, :, hh, :]
                    nc.sync.dma_start(out=dstv, in_=srcv)

            # ------- pooling -------
            ppool = ps_misc.tile([128, 512], F32, tag="pm", name="ppool")
            for t in range(MTILES):
                mr = max(0, min(128, NLOC - t * 128))
                t_o = sb.tile([128, H, 64], BF16, tag="to", name=f"to{t}")
                nc.sync.dma_start(
                    out=t_o[:].rearrange("p h d -> p (h d)"),
                    in_=o2_node[t * 128:(t + 1) * 128, :])
                t_r = sb.tile([128, 64], BF16, tag="tr", name=f"tr{t}")
                if mr < 128:
                    nc.vector.memset(t_r[:], 0.0)
                if mr > 0:
                    t_m = sb.tile([128, 64], F32, tag="tm", name=f"tm{t}")
                    nc.vector.tensor_tensor(out=t_m[:], in0=t_o[:, 0, :],
                                            in1=t_o[:, 1, :],
                                            op=mybir.AluOpType.add)
                    nc.vector.tensor_tensor(out=t_m[:], in0=t_m[:],
                                            in1=t_o[:, 2, :],
                                            op=mybir.AluOpType.add)
                    t_xr2 = sb.tile([128, DPAD], BF16, tag="txr1", name=f"txr2{t}")
                    nc.sync.dma_start(out=t_xr2[0:mr, :],
                                      in_=xr_tab[t * 128:t * 128 + mr, :])
                    t_xm = sb.tile([128, 64], F32, tag="txm", name=f"txm{t}")
                    nc.vector.tensor_tensor(out=t_xm[0:mr, :],
                                            in0=t_xr2[0:mr, 0:64],
                                            in1=t_xr2[0:mr, 64:128],
                                            op=mybir.AluOpType.add)
                    nc.vector.tensor_tensor(out=t_xm[0:mr, :], in0=t_xm[0:mr, :],
                                            in1=t_xr2[0:mr, 128:192],
                                            op=mybir.AluOpType.add)
                    nc.vector.tensor_tensor(out=t_m[0:mr, :], in0=t_m[0:mr, :],
                                            in1=t_xm[0:mr, :],
                                            op=mybir.AluOpType.subtract)
                    nc.vector.tensor_scalar_mul(t_m[0:mr, :], t_m[0:mr, :], 1.0 / 3.0)
                    nc.vector.tensor_tensor(out=t_m[0:mr, :], in0=t_m[0:mr, :],
                                            in1=t_b2[0:mr, :],
                                            op=mybir.AluOpType.add)
                    nc.vector.tensor_scalar_max(t_r[0:mr, :], t_m[0:mr, :], 0.0)
                nc.tensor.matmul(ppool[0:G, 0:64],
                                 lhsT=t_oh[:, t * G:(t + 1) * G], rhs=t_r[:],
                                 start=(t == 0), stop=(t == MTILES - 1))
            t_pl = sb.tile([G, C], F32, tag="tpl", name="t_pl")
            nc.vector.tensor_copy(out=t_pl[:], in_=ppool[0:G, 0:64])
            nc.gpsimd.dma_start(out=pool_in[:], in_=t_pl[:])
            nc.gpsimd.collective_compute(
                "AllReduce", mybir.AluOpType.add,
                replica_groups=[list(range(NCORES))],
                ins=[pool_in[:].opt()], outs=[pool_out[:].opt()])
            t_pool = sb.tile([G, C], F32, tag="tpool", name="t_pool")
            nc.gpsimd.dma_start(out=t_pool[:], in_=pool_out[:])
            t_cnt = sb.tile([G, 1], F32, tag="tcnt", name="t_cnt")
            nc.sync.dma_start(out=t_cnt[:], in_=cntr[:])
            nc.vector.tensor_scalar(out=t_pool[:], in0=t_pool[:],
                                    scalar1=t_cnt[:], scalar2=None,
                                    op0=mybir.AluOpType.mult)
            idf = cpool.tile([128, 128], F32, name="idf")
            make_identity(nc, idf[:])
            ppt = ps_misc.tile([128, 512], F32, tag="pm", name="ppt")
            nc.tensor.transpose(out=ppt[0:C, 0:G], in_=t_pool[:],
                                identity=idf[0:G, 0:G])
            t_poolT = sb.tile([C, G], F32, tag="poolT", name="t_poolT")
            nc.vector.tensor_copy(out=t_poolT[:], in_=ppt[0:C, 0:G])
            t_wc = sb.tile([C, NCLS], F32, tag="twc", name="t_wc")
            nc.sync.dma_start(out=t_wc[:], in_=wc_in[:])
            plog = ps_misc.tile([128, 512], F32, tag="pm", name="plog")
            nc.tensor.matmul(plog[0:G, 0:NCLS], lhsT=t_poolT[:], rhs=t_wc[:],
                             start=True, stop=True)
            t_bc = sb.tile([G, NCLS], F32, tag="tbc", name="t_bc")
            nc.sync.dma_start(out=t_bc[:], in_=bc_rep[:])
            t_log = sb.tile([G, NCLS], F32, tag="tlog", name="t_log")
            nc.vector.tensor_tensor(out=t_log[:], in0=plog[0:G, 0:NCLS],
                                    in1=t_bc[:], op=mybir.AluOpType.add)
            t_ex = sb.tile([G, NCLS], F32, tag="tex", name="t_ex")
            nc.scalar.activation(out=t_ex[:], in_=t_log[:],
                                 func=mybir.ActivationFunctionType.Exp)
            t_sm = sb.tile([G, 1], F32, tag="tsm", name="t_sm")
            nc.vector.tensor_reduce(out=t_sm[:], in_=t_ex[:],
                                    axis=mybir.AxisListType.X,
                                    op=mybir.AluOpType.add)
            t_rc = sb.tile([G, 1], F32, tag="trc", name="t_rc")
            nc.vector.reciprocal(out=t_rc[:], in_=t_sm[:])
            t_out = sb.tile([G, NCLS], F32, tag="tout", name="t_out")
            nc.vector.tensor_scalar(out=t_out[:], in0=t_ex[:],
                                    scalar1=t_rc[:], scalar2=None,
                                    op0=mybir.AluOpType.mult)
            nc.sync.dma_start(out=out_ext[:], in_=t_out[:])

    nc.compile()
    return nc


# ------------------------------------------------------------------
# entry point
# ------------------------------------------------------------------

def kernel(x, edge_index, batch, Wl1, Wr1, att1, b1, Wl2, Wr2, att2, b2, Wc, bc,
           _want_trace=False):
    bf = ml_dtypes.bfloat16
    x = np.asarray(x, np.float32)
    cores, meta = _preprocess(edge_index, batch)

    def padw(W):
        W = np.asarray(W, np.float32)
        return np.ascontiguousarray(
            np.pad(W, ((0, 0), (0, DPAD - W.shape[1]))).astype(bf))

    att1f = np.asarray(att1, np.float32).reshape(1, 192)
    att2f = np.asarray(att2, np.float32).reshape(1, 192)
    cnt = np.bincount(np.asarray(batch, np.int64), minlength=G).astype(np.float32)

    common = dict(
        wl1=padw(Wl1), wr1=padw(Wr1), wl2=padw(Wl2), wr2=padw(Wr2),
        att1_rep=np.ascontiguousarray(np.tile(att1f, (128, 1)).astype(bf)),
        att2_rep=np.ascontiguousarray(np.tile(att2f, (128, 1)).astype(bf)),
        b1_rep=np.ascontiguousarray(
            np.tile(np.asarray(b1, np.float32).reshape(1, 192), (128, 1)).astype(bf)),
        b2_rep=np.ascontiguousarray(
            np.tile(np.asarray(b2, np.float32).reshape(1, 64), (128, 1))),
        wc=np.ascontiguousarray(np.asarray(Wc, np.float32)),
        bc_rep=np.ascontiguousarray(
            np.tile(np.asarray(bc, np.float32).reshape(1, NCLS), (G, 1))),
        cnt_recip=np.ascontiguousarray(
            (1.0 / np.maximum(cnt, 1.0)).reshape(G, 1)),
    )

    nc = _build(meta)

    in_maps = []
    for c in range(NCORES):
        im = dict(common)
        im["xT"] = np.ascontiguousarray(x[c * NLOC:(c + 1) * NLOC, :].T.astype(bf))
        im["oh"] = _onehots(batch, c)
        im["xl_idx16"] = cores[c]["xl_idx16"]
        im["xr_idx16"] = cores[c]["xr_idx16"]
        im["sel01"] = cores[c]["sel01"]
        in_maps.append(im)

    res = bass_utils.run_bass_kernel_spmd(
        nc, in_maps, core_ids=list(range(NCORES)), trace=_want_trace)
    out = np.asarray(res.results[0]["out"], np.float32)
    kernel._last_exec_ns = getattr(res, "exec_time_ns", None)
    return out


# revision 13
# speedup vs baseline: 2.2572x; 2.2572x over previous
"""GATv2 2-layer GNN + global mean pool on 8 TRN2 NeuronCores (Bass/Tile).

Host: graph partitioning + metadata in numpy. Device: SPMD kernel on cores
0-7 via run_bass_kernel_spmd. See transcript design notes.

Sharding: core c owns nodes [c*6250, (c+1)*6250) and all edges whose dst is
in that range (self-loops included). Per layer, each core computes its
xl = x@Wl shard (bf16 rows padded to 256 cols), AllGathers the full table,
keeps xr = x@Wr local (row col 192 = 1.0 for the softmax denominator).
Per-edge s = xl[src]+xr[dst] via two bulk dma_gathers + DVE add; leaky_relu
via fused scalar_tensor_tensor; per-head logits via att-broadcast multiply +
strided tree reduce; exp on ScalarE; segment softmax + aggregation fused into
per-chunk TensorE matmuls (lhsT = Sel01*exp) accumulating into a PSUM node
grid (40 nodes x 3 heads = 120 rows per bank, 4 banks = 160 nodes per set).
Normalize by 1/den, write slot-major, reshape to node-major via affine DMAs,
apply -xr + b, relu. Pool via one-hot matmuls + AllReduce; final linear +
softmax replicated on every core.
"""
import sys

sys.path.insert(0, "/opt/trn_rl_repo")

import numpy as np
import ml_dtypes

import concourse.bass as bass
import concourse.mybir as mybir
import concourse.tile as tile
import concourse.bacc as bacc
from concourse import bass_utils
from concourse.masks import make_identity

BF16 = mybir.dt.bfloat16
F32 = mybir.dt.float32
I16 = mybir.dt.int16

N, E, F, H, C, G, NCLS = 50000, 800000, 128, 3, 64, 16, 10
NCORES = 8
NLOC = N // NCORES            # 6250
HALF = N // 2                 # 25000
BANK_NODES = 40               # nodes per PSUM bank -> m = 120
NBANKS = 4                    # PSUM banks used by aggregation per set
SET_NODES = BANK_NODES * NBANKS   # 160
NSETS = -(-NLOC // SET_NODES)     # 40
DPAD = 256                    # padded table row (bf16) -> 512B
DW = 193                      # 192 feats + ones col
NPAD = NSETS * SET_NODES      # 6400
MTILES = -(-NPAD // 128)      # 50


# ------------------------------------------------------------------
# host preprocessing
# ------------------------------------------------------------------

def _wrap16(seq):
    n = seq.size
    w = np.asarray(seq, np.int16).reshape(n // 16, 16).T
    return np.ascontiguousarray(np.tile(w, (8, 1)))


def _preprocess(edge_index, batch):
    src_g = np.concatenate([np.asarray(edge_index[0]), np.arange(N, dtype=np.int64)])
    dst_g = np.concatenate([np.asarray(edge_index[1]), np.arange(N, dtype=np.int64)])

    per_core = []
    counts = np.zeros((NCORES, NSETS, NBANKS, 2), np.int64)
    for c in range(NCORES):
        m = (dst_g // NLOC) == c
        src = src_g[m].astype(np.int64)
        dst = (dst_g[m] - c * NLOC).astype(np.int64)
        half = (src >= HALF).astype(np.int64)
        order = np.argsort(dst * 2 + half, kind="stable")
        src, dst, half = src[order], dst[order], half[order]
        bank_id = dst // BANK_NODES
        set_id = bank_id // NBANKS
        bank = bank_id % NBANKS
        np.add.at(counts[c], (set_id, bank, half), 1)
        per_core.append((src, dst, set_id, bank, half))

    kch = np.maximum(1, -(-counts.max(axis=0) // 128))   # [NSETS, NBANKS, 2]
    reg_ch = kch.sum(axis=1)                             # chunks per (set, half)
    reg_ch += reg_ch % 2                                 # 256-slot alignment
    tot_ch = int(reg_ch.sum())
    tot_slots = tot_ch * 128

    jobs = [[] for _ in range(NSETS)]
    chunk_meta = []          # global chunk -> (set, half, bank or -1)
    set_nch = []
    reg_off = []
    pos_slots = 0
    for k in range(NSETS):
        col = 0
        first = [True] * NBANKS
        offs = []
        for hf in range(2):
            a = pos_slots
            used = 0
            for b in range(NBANKS):
                for _ in range(int(kch[k, b, hf])):
                    jobs[k].append([col, b, first[b], False])
                    first[b] = False
                    chunk_meta.append((k, hf, b))
                    col += 1
                    used += 1
            while used < int(reg_ch[k, hf]):
                chunk_meta.append((k, hf, -1))
                col += 1
                used += 1
            pos_slots += int(reg_ch[k, hf]) * 128
            offs.append((a, pos_slots))
        lastj = {}
        for j, jb in enumerate(jobs[k]):
            lastj[jb[1]] = j
        for b, j in lastj.items():
            jobs[k][j][3] = True
        set_nch.append(col)
        reg_off.append(offs)
    assert sum(set_nch) == tot_ch
    assert pos_slots == tot_slots

    cores = []
    for c in range(NCORES):
        src, dst, set_id, bank, half = per_core[c]
        xl_idx = np.zeros(tot_slots, np.int64)
        xr_idx = np.zeros(tot_slots, np.int64)
        selpat = np.full(tot_slots, -1, np.int64)
        cursor = {}
        pos = 0
        for (k, hf, b) in chunk_meta:
            if b >= 0:
                key = (k, b, hf)
                if key not in cursor:
                    selm = (set_id == k) & (bank == b) & (half == hf)
                    cursor[key] = [src[selm], dst[selm], 0]
                es_all, ed_all, cpos = cursor[key]
                n = min(128, es_all.size - cpos)
                es = es_all[cpos:cpos + n]
                ed = ed_all[cpos:cpos + n]
                cursor[key][2] = cpos + n
                sl = slice(pos, pos + n)
                xl_idx[sl] = es - HALF * hf
                xr_idx[sl] = ed
                selpat[sl] = ed % BANK_NODES
            pos += 128
        for key, (es_all, ed_all, cpos) in cursor.items():
            assert cpos == es_all.size, (c, key, cpos, es_all.size)

        sel = np.zeros((128, tot_ch, 120), ml_dtypes.bfloat16)
        sp = selpat.reshape(tot_ch, 128)
        ii, jj = (sp >= 0).nonzero()     # ii = chunk, jj = row
        for hh in range(H):
            sel[jj, ii, BANK_NODES * hh + sp[ii, jj]] = 1.0
        cores.append(dict(
            xl_idx16=_wrap16(xl_idx), xr_idx16=_wrap16(xr_idx),
            sel01=np.ascontiguousarray(sel.reshape(128, tot_ch * 120))))

    meta = dict(jobs=jobs, set_nch=set_nch, reg_off=reg_off,
                tot_ch=tot_ch, tot_slots=tot_slots)
    return cores, meta


def _onehots(batch, core):
    oh = np.zeros((128, MTILES, G), ml_dtypes.bfloat16)
    base = core * NLOC
    bat = np.asarray(batch, np.int64)
    for t in range(MTILES):
        n0 = t * 128
        n1 = min(n0 + 128, NLOC)
        if n1 > n0:
            rows = np.arange(n0, n1)
            oh[rows - n0, t, bat[base + rows]] = 1.0
    return np.ascontiguousarray(oh.reshape(128, MTILES * G))


# ------------------------------------------------------------------
# device builder
# ------------------------------------------------------------------

def _build(meta):
    nc = bacc.Bacc()
    jobs, set_nch, reg_off = meta["jobs"], meta["set_nch"], meta["reg_off"]
    tot_ch, tot_slots = meta["tot_ch"], meta["tot_slots"]

    xT = nc.declare_dram_parameter("xT", [F, NLOC], BF16, isOutput=False)
    wl1 = nc.declare_dram_parameter("wl1", [F, DPAD], BF16, isOutput=False)
    wr1 = nc.declare_dram_parameter("wr1", [F, DPAD], BF16, isOutput=False)
    wl2 = nc.declare_dram_parameter("wl2", [192, DPAD], BF16, isOutput=False)
    wr2 = nc.declare_dram_parameter("wr2", [192, DPAD], BF16, isOutput=False)
    att1_rep = nc.declare_dram_parameter("att1_rep", [128, 192], BF16, isOutput=False)
    att2_rep = nc.declare_dram_parameter("att2_rep", [128, 192], BF16, isOutput=False)
    b1_rep = nc.declare_dram_parameter("b1_rep", [128, 192], BF16, isOutput=False)
    b2_rep = nc.declare_dram_parameter("b2_rep", [128, 64], F32, isOutput=False)
    wc_in = nc.declare_dram_parameter("wc", [C, NCLS], F32, isOutput=False)
    bc_rep = nc.declare_dram_parameter("bc_rep", [G, NCLS], F32, isOutput=False)
    cntr = nc.declare_dram_parameter("cnt_recip", [G, 1], F32, isOutput=False)
    oh_in = nc.declare_dram_parameter("oh", [128, MTILES * G], BF16, isOutput=False)
    xl_idx = nc.declare_dram_parameter("xl_idx16", [128, tot_slots // 16], I16, isOutput=False)
    xr_idx = nc.declare_dram_parameter("xr_idx16", [128, tot_slots // 16], I16, isOutput=False)
    sel_in = nc.declare_dram_parameter("sel01", [128, tot_ch * 120], BF16, isOutput=False)
    out_ext = nc.declare_dram_parameter("out", [G, NCLS], F32, isOutput=True)

    shard_tab = nc.dram_tensor("shard_tab", [NLOC, DPAD], BF16)
    glob_tab = nc.dram_tensor("glob_tab", [N, DPAD], BF16)
    glob_hi = nc.dram_tensor("glob_hi", [HALF, DPAD], BF16)
    xr_tab = nc.dram_tensor("xr_tab", [NLOC, DPAD], BF16)
    h_slots = nc.dram_tensor("h_slots", [NSETS, 128, NBANKS * DW], BF16)
    h1_node = nc.dram_tensor("h1_node", [NPAD, 192], BF16)
    o2_node = nc.dram_tensor("o2_node", [NPAD, H * 64], BF16)
    pool_in = nc.dram_tensor("pool_in", [G, C], F32)
    pool_out = nc.dram_tensor("pool_out", [G, C], F32)

    with nc.allow_low_precision(reason="bf16 tree-reduce + staging validated within 2e-2 tolerance"), tile.TileContext(nc) as tc:
        with (
            tc.tile_pool(name="const", bufs=1) as cpool,
            tc.tile_pool(name="sbuf", bufs=2) as sb,
            tc.tile_pool(name="agg", bufs=1, space="PSUM") as ps_agg,
            tc.tile_pool(name="ptf", bufs=1, space="PSUM") as ps_tf,
            tc.tile_pool(name="pmisc", bufs=1, space="PSUM") as ps_misc,
            tc.tile_pool(name="big", bufs=1) as mp,
        ):
            t_att1 = cpool.tile([128, 192], BF16, name="t_att1")
            t_att2 = cpool.tile([128, 192], BF16, name="t_att2")
            t_b1 = cpool.tile([128, 192], BF16, name="t_b1")
            t_b2 = cpool.tile([128, 64], F32, name="t_b2")
            t_oh = cpool.tile([128, MTILES * G], BF16, name="t_oh")
            ident = cpool.tile([128, 128], BF16, name="ident")
            nc.sync.dma_start(out=t_att1[:], in_=att1_rep[:])
            nc.sync.dma_start(out=t_att2[:], in_=att2_rep[:])
            nc.sync.dma_start(out=t_b1[:], in_=b1_rep[:])
            nc.sync.dma_start(out=t_b2[:], in_=b2_rep[:])
            nc.sync.dma_start(out=t_oh[:], in_=oh_in[:])
            make_identity(nc, ident[:])

            # ------- layer-1 transforms -------
            t_xT = mp.tile([128, NLOC], BF16, name="t_xT")
            nc.sync.dma_start(out=t_xT[:], in_=xT[:])
            t_wl = cpool.tile([128, DPAD], BF16, name="t_wl")
            t_wr = cpool.tile([128, DPAD], BF16, name="t_wr")
            nc.sync.dma_start(out=t_wl[:], in_=wl1[:])
            nc.sync.dma_start(out=t_wr[:], in_=wr1[:])

            ntile = -(-NLOC // 128)
            for t in range(ntile):
                mr = min(128, NLOC - t * 128)
                for which, (wt, dtab) in enumerate(((t_wl, shard_tab), (t_wr, xr_tab))):
                    pst = ps_tf.tile([128, DPAD], F32, tag="tf", name=f"p1_{t}_{which}")
                    nc.tensor.matmul(pst[0:mr, :],
                                     lhsT=t_xT[:, t * 128:t * 128 + mr],
                                     rhs=wt[:], start=True, stop=True)
                    stg = sb.tile([128, DPAD], BF16, tag="tfs", name=f"s1_{t}_{which}")
                    nc.scalar.copy(out=stg[0:mr, :], in_=pst[0:mr, :])
                    if which == 1:
                        nc.vector.memset(stg[0:mr, 192:193], 1.0)
                    nc.sync.dma_start(out=dtab[t * 128:t * 128 + mr, :],
                                      in_=stg[0:mr, :])

            nc.gpsimd.collective_compute(
                "AllGather", mybir.AluOpType.bypass,
                replica_groups=[list(range(NCORES))],
                ins=[shard_tab[:].opt()], outs=[glob_tab[:].opt()])
            nc.sync.dma_start(out=glob_hi[:], in_=glob_tab[HALF:N, :])

            # ------- edge layer -------
            def edge_layer(layer, t_att):
                for k in range(NSETS):
                    nch = set_nch[k]
                    (lo_a, lo_b), (hi_a, hi_b) = reg_off[k]
                    nsl = nch * 128
                    nlo = lo_b - lo_a
                    nhi = hi_b - hi_a
                    ti_xl = sb.tile([128, nsl // 16], I16, tag="ixl", name=f"ixl{layer}_{k}")
                    ti_xr = sb.tile([128, nsl // 16], I16, tag="ixr", name=f"ixr{layer}_{k}")
                    nc.sync.dma_start(out=ti_xl[:],
                                      in_=xl_idx[:, lo_a // 16:lo_a // 16 + nsl // 16])
                    nc.sync.dma_start(out=ti_xr[:],
                                      in_=xr_idx[:, lo_a // 16:lo_a // 16 + nsl // 16])
                    g_xl = sb.tile([128, nch, DPAD], BF16, tag="gxl", name=f"gxl{layer}_{k}", bufs=3)
                    g_xr = sb.tile([128, nch, DPAD], BF16, tag="gxr", name=f"gxr{layer}_{k}")
                    if nlo > 0:
                        nc.gpsimd.dma_gather(
                            out_ap=g_xl[:, 0:nlo // 128, :],
                            in_ap=glob_tab[0:HALF, :],
                            idxs_ap=ti_xl[:, 0:nlo // 16],
                            num_idxs=nlo, num_idxs_reg=nlo, elem_size=DPAD, single_packet=False)
                    if nhi > 0:
                        nc.gpsimd.dma_gather(
                            out_ap=g_xl[:, nlo // 128:nch, :],
                            in_ap=glob_hi[:],
                            idxs_ap=ti_xl[:, nlo // 16:nsl // 16],
                            num_idxs=nhi, num_idxs_reg=nhi, elem_size=DPAD, single_packet=False)
                    nc.gpsimd.dma_gather(
                        out_ap=g_xr[:], in_ap=xr_tab[:], idxs_ap=ti_xr[:],
                        num_idxs=nsl, num_idxs_reg=nsl, elem_size=DPAD, single_packet=False)
                    t_s = sb.tile([128, nch, DW], BF16, tag="ts", name=f"ts{layer}_{k}")
                    nc.vector.tensor_tensor(out=t_s[:], in0=g_xl[:, :, 0:DW],
                                            in1=g_xr[:, :, 0:DW],
                                            op=mybir.AluOpType.add)
                    t_lk = sb.tile([128, nch, 192], BF16, tag="tlk", name=f"tlk{layer}_{k}")
                    nc.vector.scalar_tensor_tensor(
                        out=t_lk[:], in0=t_s[:, :, 0:192], scalar=0.2,
                        in1=t_s[:, :, 0:192],
                        op0=mybir.AluOpType.mult, op1=mybir.AluOpType.max)
                    att_b = bass.AP(t_att[:].tensor, t_att[:].offset,
                                    [list(t_att[:].ap[0]), [0, nch], [1, 192]])
                    nc.vector.tensor_tensor(out=t_lk[:], in0=t_lk[:], in1=att_b,
                                            op=mybir.AluOpType.mult)
                    v = t_lk[:].rearrange("p c (h w) -> p c h w", h=H)
                    w = 32
                    while w > 1:
                        nc.vector.tensor_tensor(out=v[:, :, :, 0:w],
                                                in0=v[:, :, :, 0:w],
                                                in1=v[:, :, :, w:2 * w],
                                                op=mybir.AluOpType.add)
                        w //= 2
                    t_lg = sb.tile([128, nch, H], F32, tag="tlg", name=f"tlg{layer}_{k}")
                    nc.vector.tensor_tensor(out=t_lg[:],
                                            in0=v[:, :, :, 0:1].squeeze(3),
                                            in1=v[:, :, :, 1:2].squeeze(3),
                                            op=mybir.AluOpType.add)
                    t_e = sb.tile([128, nch, H], BF16, tag="te", name=f"te{layer}_{k}")
                    nc.scalar.activation(out=t_e[:], in_=t_lg[:],
                                         func=mybir.ActivationFunctionType.Exp)
                    t_sel = sb.tile([128, nch, 120], BF16, tag="tsel", name=f"tsel{layer}_{k}")
                    ch0 = sum(set_nch[:k])
                    nc.sync.dma_start(out=t_sel[:],
                                      in_=sel_in[:, ch0 * 120:(ch0 + nch) * 120])
                    eb = bass.AP(t_e[:].tensor, t_e[:].offset,
                                 [list(t_e[:].ap[0]), [H, nch], [1, H], [0, BANK_NODES]])
                    nc.vector.tensor_tensor(
                        out=t_sel[:].rearrange("p c (h s) -> p c h s", h=H),
                        in0=t_sel[:].rearrange("p c (h s) -> p c h s", h=H),
                        in1=eb, op=mybir.AluOpType.mult)
                    pagg = ps_agg.tile([128, NBANKS * 512], F32, tag="pagg",
                                       name=f"pagg{layer}_{k}")
                    for (col, b, st, sp_) in jobs[k]:
                        nc.tensor.matmul(
                            pagg[0:120, b * 512:b * 512 + DW],
                            lhsT=t_sel[:, col, :],
                            rhs=t_s[:, col, :],
                            start=st, stop=sp_)
                    t_ev = sb.tile([128, NBANKS, DW], BF16, tag="tev", name=f"tev{layer}_{k}")
                    pagg_v = bass.AP(pagg[:].tensor, pagg[:].offset,
                                     [list(pagg[:].ap[0]), [512, NBANKS], [1, DW]])
                    nc.scalar.copy(out=t_ev[:], in_=pagg_v)
                    t_d = sb.tile([128, NBANKS], BF16, tag="td", name=f"td{layer}_{k}")
                    nc.vector.reciprocal(out=t_d[:], in_=t_ev[:, :, 192:193].squeeze(2))
                    db = bass.AP(t_d[:].tensor, t_d[:].offset,
                                 [list(t_d[:].ap[0]), [1, NBANKS], [0, DW]])
                    nc.vector.tensor_tensor(out=t_ev[:], in0=t_ev[:], in1=db,
                                            op=mybir.AluOpType.mult)
                    nc.sync.dma_start(out=h_slots[k, :, :],
                                      in_=t_ev[:].rearrange("p b d -> p (b d)"))

            edge_layer(1, t_att1)

            # ------- reshape slots -> node-major (layer 1) -------
            # slot row 40h+s of (set k, bank b) -> node k*160 + b*40 + s,
            # cols [64h, 64h+64)
            for hh in range(H):
                for b in range(NBANKS):
                    srcv = h_slots[:, 40 * hh:40 * hh + 40,
                                   b * DW + 64 * hh:b * DW + 64 * hh + 64]
                    dstv = h1_node[:].rearrange(
                        "(k b s) d -> k b s d", k=NSETS, b=NBANKS)[
                        :, b, :, 64 * hh:64 * hh + 64]
                    nc.sync.dma_start(out=dstv, in_=srcv)

            # ------- h1 = relu(slots/den - xr1 + b1); build h1T planes -------
            t_wl2a = cpool.tile([128, DPAD], BF16, name="t_wl2a")
            t_wl2b = cpool.tile([64, DPAD], BF16, name="t_wl2b")
            t_wr2a = cpool.tile([128, DPAD], BF16, name="t_wr2a")
            t_wr2b = cpool.tile([64, DPAD], BF16, name="t_wr2b")
            nc.sync.dma_start(out=t_wl2a[:], in_=wl2[0:128, :])
            nc.sync.dma_start(out=t_wl2b[:], in_=wl2[128:192, :])
            nc.sync.dma_start(out=t_wr2a[:], in_=wr2[0:128, :])
            nc.sync.dma_start(out=t_wr2b[:], in_=wr2[128:192, :])
            h1T_a = mp.tile([128, NPAD], BF16, name="h1T_a")
            h1T_b = mp.tile([64, NPAD], BF16, name="h1T_b")

            for t in range(MTILES):
                mr = max(0, min(128, NLOC - t * 128))
                t_h = sb.tile([128, 192], BF16, tag="th", name=f"th{t}")
                nc.sync.dma_start(out=t_h[:], in_=h1_node[t * 128:(t + 1) * 128, :])
                th2 = sb.tile([128, 192], BF16, tag="th2", name=f"th2{t}")
                if mr < 128:
                    nc.vector.memset(th2[:], 0.0)
                if mr > 0:
                    t_xr1 = sb.tile([128, DPAD], BF16, tag="txr1", name=f"txr1{t}")
                    nc.sync.dma_start(out=t_xr1[0:mr, :],
                                      in_=xr_tab[t * 128:t * 128 + mr, :])
                    nc.vector.tensor_tensor(out=th2[0:mr, :], in0=t_h[0:mr, :],
                                            in1=t_xr1[0:mr, 0:192],
                                            op=mybir.AluOpType.subtract)
                    nc.vector.tensor_tensor(out=th2[0:mr, :], in0=th2[0:mr, :],
                                            in1=t_b1[0:mr, :],
                                            op=mybir.AluOpType.add)
                    nc.vector.tensor_scalar_max(th2[0:mr, :], th2[0:mr, :], 0.0)
                for h2 in range(2):
                    wdt = 128 if h2 == 0 else 64
                    ptr = ps_misc.tile([128, 512], BF16, tag="pm", name=f"ptr{t}_{h2}")
                    nc.tensor.transpose(out=ptr[0:wdt, 0:128],
                                        in_=th2[:, h2 * 128:h2 * 128 + wdt],
                                        identity=ident[:])
                    dst = h1T_a if h2 == 0 else h1T_b
                    nc.vector.tensor_copy(out=dst[0:wdt, t * 128:(t + 1) * 128],
                                          in_=ptr[0:wdt, 0:128])

            # ------- layer-2 transforms -------
            for t in range(MTILES):
                mr = max(0, min(128, NLOC - t * 128))
                if mr == 0:
                    continue
                for which, (wta, wtb, dtab) in enumerate((
                        (t_wl2a, t_wl2b, shard_tab), (t_wr2a, t_wr2b, xr_tab))):
                    pst = ps_tf.tile([128, DPAD], F32, tag="tf", name=f"p2_{t}_{which}")
                    nc.tensor.matmul(pst[0:mr, :],
                                     lhsT=h1T_a[:, t * 128:t * 128 + mr],
                                     rhs=wta[:], start=True, stop=False)
                    nc.tensor.matmul(pst[0:mr, :],
                                     lhsT=h1T_b[:, t * 128:t * 128 + mr],
                                     rhs=wtb[:], start=False, stop=True)
                    stg = sb.tile([128, DPAD], BF16, tag="tfs", name=f"s2_{t}_{which}")
                    nc.scalar.copy(out=stg[0:mr, :], in_=pst[0:mr, :])
                    if which == 1:
                        nc.vector.memset(stg[0:mr, 192:193], 1.0)
                    nc.sync.dma_start(out=dtab[t * 128:t * 128 + mr, :],
                                      in_=stg[0:mr, :])

            nc.gpsimd.collective_compute(
                "AllGather", mybir.AluOpType.bypass,
                replica_groups=[list(range(NCORES))],
                ins=[shard_tab[:].opt()], outs=[glob_tab[:].opt()])
            nc.sync.dma_start(out=glob_hi[:], in_=glob_tab[HALF:N, :])

            edge_layer(2, t_att2)

            for hh in range(H):
                for b in range(NBANKS):
                    srcv = h_slots[:, 40 * hh:40 * hh + 40,
                                   b * DW + 64 * hh:b * DW + 64 * hh + 64]
                    dstv = o2_node[:].rearrange(
                        "(k b s) (h d) -> k b s h d", k=NSETS, b=NBANKS, h=H)[
                        :, b, :, hh, :]
                    nc.sync.dma_start(out=dstv, in_=srcv)

            # ------- pooling -------
            ppool = ps_misc.tile([128, 512], F32, tag="pm", name="ppool")
            for t in range(MTILES):
                mr = max(0, min(128, NLOC - t * 128))
                t_o = sb.tile([128, H, 64], BF16, tag="to", name=f"to{t}")
                nc.sync.dma_start(
                    out=t_o[:].rearrange("p h d -> p (h d)"),
                    in_=o2_node[t * 128:(t + 1) * 128, :])
                t_r = sb.tile([128, 64], BF16, tag="tr", name=f"tr{t}")
                if mr < 128:
                    nc.vector.memset(t_r[:], 0.0)
                if mr > 0:
                    t_m = sb.tile([128, 64], F32, tag="tm", name=f"tm{t}")
                    nc.vector.tensor_tensor(out=t_m[:], in0=t_o[:, 0, :],
                                            in1=t_o[:, 1, :],
                                            op=mybir.AluOpType.add)
                    nc.vector.tensor_tensor(out=t_m[:], in0=t_m[:],
                                            in1=t_o[:, 2, :],
                                            op=mybir.AluOpType.add)
                    t_xr2 = sb.tile([128, DPAD], BF16, tag="txr1", name=f"txr2{t}")
                    nc.sync.dma_start(out=t_xr2[0:mr, :],
                                      in_=xr_tab[t * 128:t * 128 + mr, :])
                    t_xm = sb.tile([128, 64], F32, tag="txm", name=f"txm{t}")
                    nc.vector.tensor_tensor(out=t_xm[0:mr, :],
                                            in0=t_xr2[0:mr, 0:64],
                                            in1=t_xr2[0:mr, 64:128],
                                            op=mybir.AluOpType.add)
                    nc.vector.tensor_tensor(out=t_xm[0:mr, :], in0=t_xm[0:mr, :],
                                            in1=t_xr2[0:mr, 128:192],
                                            op=mybir.AluOpType.add)
                    nc.vector.tensor_tensor(out=t_m[0:mr, :], in0=t_m[0:mr, :],
                                            in1=t_xm[0:mr, :],
                                            op=mybir.AluOpType.subtract)
                    nc.vector.tensor_scalar_mul(t_m[0:mr, :], t_m[0:mr, :], 1.0 / 3.0)
                    nc.vector.tensor_tensor(out=t_m[0:mr, :], in0=t_m[0:mr, :],
                                            in1=t_b2[0:mr, :],
                                            op=mybir.AluOpType.add)
                    nc.vector.tensor_scalar_max(t_r[0:mr, :], t_m[0:mr, :], 0.0)
                nc.tensor.matmul(ppool[0:G, 0:64],
                                 lhsT=t_oh[:, t * G:(t + 1) * G], rhs=t_r[:],
                                 start=(t == 0), stop=(t == MTILES - 1))
            t_pl = sb.tile([G, C], F32, tag="tpl", name="t_pl")
            nc.vector.tensor_copy(out=t_pl[:], in_=ppool[0:G, 0:64])
            nc.gpsimd.dma_start(out=pool_in[:], in_=t_pl[:])
            nc.gpsimd.collective_compute(
                "AllReduce", mybir.AluOpType.add,
                replica_groups=[list(range(NCORES))],
                ins=[pool_in[:].opt()], outs=[pool_out[:].opt()])
            t_pool = sb.tile([G, C], F32, tag="tpool", name="t_pool")
            nc.gpsimd.dma_start(out=t_pool[:], in_=pool_out[:])
            t_cnt = sb.tile([G, 1], F32, tag="tcnt", name="t_cnt")
            nc.sync.dma_start(out=t_cnt[:], in_=cntr[:])
            nc.vector.tensor_scalar(out=t_pool[:], in0=t_pool[:],
                                    scalar1=t_cnt[:], scalar2=None,
                                    op0=mybir.AluOpType.mult)
            idf = cpool.tile([128, 128], F32, name="idf")
            make_identity(nc, idf[:])
            ppt = ps_misc.tile([128, 512], F32, tag="pm", name="ppt")
            nc.tensor.transpose(out=ppt[0:C, 0:G], in_=t_pool[:],
                                identity=idf[0:G, 0:G])
            t_poolT = sb.tile([C, G], F32, tag="poolT", name="t_poolT")
            nc.vector.tensor_copy(out=t_poolT[:], in_=ppt[0:C, 0:G])
            t_wc = sb.tile([C, NCLS], F32, tag="twc", name="t_wc")
            nc.sync.dma_start(out=t_wc[:], in_=wc_in[:])
            plog = ps_misc.tile([128, 512], F32, tag="pm", name="plog")
            nc.tensor.matmul(plog[0:G, 0:NCLS], lhsT=t_poolT[:], rhs=t_wc[:],
                             start=True, stop=True)
            t_bc = sb.tile([G, NCLS], F32, tag="tbc", name="t_bc")
            nc.sync.dma_start(out=t_bc[:], in_=bc_rep[:])
            t_log = sb.tile([G, NCLS], F32, tag="tlog", name="t_log")
            nc.vector.tensor_tensor(out=t_log[:], in0=plog[0:G, 0:NCLS],
                                    in1=t_bc[:], op=mybir.AluOpType.add)
            t_ex = sb.tile([G, NCLS], F32, tag="tex", name="t_ex")
            nc.scalar.activation(out=t_ex[:], in_=t_log[:],
                                 func=mybir.ActivationFunctionType.Exp)
            t_sm = sb.tile([G, 1], F32, tag="tsm", name="t_sm")
            nc.vector.tensor_reduce(out=t_sm[:], in_=t_ex[:],
                                    axis=mybir.AxisListType.X,
                                    op=mybir.AluOpType.add)
            t_rc = sb.tile([G, 1], F32, tag="trc", name="t_rc")
            nc.vector.reciprocal(out=t_rc[:], in_=t_sm[:])
            t_out = sb.tile([G, NCLS], F32, tag="tout", name="t_out")
            nc.vector.tensor_scalar(out=t_out[:], in0=t_ex[:],
                                    scalar1=t_rc[:], scalar2=None,
                                    op0=mybir.AluOpType.mult)
            nc.sync.dma_start(out=out_ext[:], in_=t_out[:])

    nc.compile()
    return nc


# ------------------------------------------------------------------
# entry point
# ------------------------------------------------------------------

def kernel(x, edge_index, batch, Wl1, Wr1, att1, b1, Wl2, Wr2, att2, b2, Wc, bc,
           _want_trace=False):
    bf = ml_dtypes.bfloat16
    x = np.asarray(x, np.float32)
    cores, meta = _preprocess(edge_index, batch)

    def padw(W):
        W = np.asarray(W, np.float32)
        return np.ascontiguousarray(
            np.pad(W, ((0, 0), (0, DPAD - W.shape[1]))).astype(bf))

    att1f = np.asarray(att1, np.float32).reshape(1, 192)
    att2f = np.asarray(att2, np.float32).reshape(1, 192)
    cnt = np.bincount(np.asarray(batch, np.int64), minlength=G).astype(np.float32)

    common = dict(
        wl1=padw(Wl1), wr1=padw(Wr1), wl2=padw(Wl2), wr2=padw(Wr2),
        att1_rep=np.ascontiguousarray(np.tile(att1f, (128, 1)).astype(bf)),
        att2_rep=np.ascontiguousarray(np.tile(att2f, (128, 1)).astype(bf)),
        b1_rep=np.ascontiguousarray(
            np.tile(np.asarray(b1, np.float32).reshape(1, 192), (128, 1)).astype(bf)),
        b2_rep=np.ascontiguousarray(
            np.tile(np.asarray(b2, np.float32).reshape(1, 64), (128, 1))),
        wc=np.ascontiguousarray(np.asarray(Wc, np.float32)),
        bc_rep=np.ascontiguousarray(
            np.tile(np.asarray(bc, np.float32).reshape(1, NCLS), (G, 1))),
        cnt_recip=np.ascontiguousarray(
            (1.0 / np.maximum(cnt, 1.0)).reshape(G, 1)),
    )

    nc = _build(meta)

    in_maps = []
    for c in range(NCORES):
        im = dict(common)
        im["xT"] = np.ascontiguousarray(x[c * NLOC:(c + 1) * NLOC, :].T.astype(bf))
        im["oh"] = _onehots(batch, c)
        im["xl_idx16"] = cores[c]["xl_idx16"]
        im["xr_idx16"] = cores[c]["xr_idx16"]
        im["sel01"] = cores[c]["sel01"]
        in_maps.append(im)

    res = bass_utils.run_bass_kernel_spmd(
        nc, in_maps, core_ids=list(range(NCORES)), trace=_want_trace)
    out = np.asarray(res.results[0]["out"], np.float32)
    kernel._last_exec_ns = getattr(res, "exec_time_ns", None)
    return out
